# revision 1
# baseline (speedup 1.0000x reference)
"""GCN message-passing kernel for Trainium2, 8 NeuronCores (SPMD).

Strategy (graph-parallel):
- Nodes are protein-contiguous, sharded across 8 cores at protein boundaries
  (16 proteins/core, node count padded to 6400/core).
- Within a core, nodes are re-ordered (bin-packed) into 100 blocks of 64 so
  each block has a near-equal number of incoming edges; the per-block edge
  lists (split "lo"/"hi" by source-core for int16 gather indexing) are padded
  to a uniform tile count so one SPMD program fits all cores.
- Per layer: hw = h @ W (fp32, feature-major), PE-transposed to node-major,
  scaled by dis=1/sqrt(deg+1) and cast bf16 in one DVE op, AllGather'd into
  a partition-major [51200,128] bf16 table; messages are fetched with
  dma_gather (single_packet=False) and segment-summed into PSUM via one-hot
  matmuls (S resident in SBUF, bf16, with dis[dst] folded into its values);
  epilogue is a single ACT relu+bias per 64-node block. Self-loops are
  regular edges (dis^2 * hw == dis * hws[self]).
- Readout (all in packed pi order, no permutation): scores via matvec
  matmuls, global-shift masked softmax (shift-invariant), PE-transposes to
  node-major, then one fused bf16 pooling matmul per 128-node tile over
  rhs = [h | ex*h | ex] (mean pool, attention numerator, denominator at
  once), final projection to [64,16] per core.
"""
import os
import numpy as np
import ml_dtypes

DBG_LAYERS = int(os.environ.get("GCN_DBG_LAYERS", "4"))
DBG_NO_COLL = os.environ.get("GCN_DBG_NO_COLL", "") == "1"
DBG_NO_GATHER = os.environ.get("GCN_DBG_NO_GATHER", "") == "1"
DBG_NO_READOUT = os.environ.get("GCN_DBG_NO_READOUT", "") == "1"
DBG_DUMP_H = os.environ.get("GCN_DBG_DUMP_H", "") == "1"

import concourse.bacc as bacc
import concourse.tile as tile
import concourse.tile_utils as tile_utils
from concourse import mybir
from concourse.bass_utils import run_bass_kernel_spmd
from concourse.masks import make_identity

bf16 = ml_dtypes.bfloat16
AF = mybir.ActivationFunctionType

NC = 8
D = 128
L = 4
B = 128
PPC = B // NC          # proteins per core
NPAD = 6400            # padded nodes per core
NPADG = NC * NPAD      # global padded rows
LO_BOUND = 32000       # lo gather covers rows [0, 32000)
HI_BASE = 18560        # hi gather covers rows [18560, 51200): 32640 <= int16
LO_CAP = 4 * 128       # target lo slots per block (flex edges spill to hi)
TW = 64                # one-hot tile width (nodes per agg block)
NBLK = NPAD // TW      # 100 agg blocks
NTB = NPAD // 128      # 50 transpose/epilogue chunks
PPAD = 512             # protein padding for softmax/pool layout
QP = PPC * PPAD        # 8192 padded-q slots
QTB = QP // 128        # 64 node-major tiles in q space
GCH = 8192             # gather slots per dma_gather instruction
NEG = -1.0e30

f32 = mybir.dt.float32
bft = mybir.dt.bfloat16
i16 = mybir.dt.int16


# ---------------------------------------------------------------- host prep

def _pack_idx(vals, slots):
    """int16 gather index layout: position i -> partition i%16, col i//16,
    replicated across the 128 partitions."""
    assert len(vals) == slots and slots % 16 == 0
    arr = np.asarray(vals, np.int16).reshape(slots // 16, 16).T  # [16, s//16]
    return np.ascontiguousarray(np.tile(arr, (8, 1)))


def _host_prep(x, edge_index, batch, lysine_mask):
    N = x.shape[0]
    src = np.asarray(edge_index[0], np.int64)
    dst = np.asarray(edge_index[1], np.int64)
    batch = np.asarray(batch, np.int64)

    pcounts = np.bincount(batch, minlength=B)       # nodes per protein
    pstart = np.concatenate([[0], np.cumsum(pcounts)])
    cstart = pstart[np.arange(NC) * PPC]            # first node of core c
    cend = pstart[(np.arange(NC) + 1) * PPC]
    ncore = cend - cstart
    assert ncore.max() <= NPAD, f"core node count {ncore.max()} > {NPAD}"
    assert pcounts.max() <= PPAD, f"protein size {pcounts.max()} > {PPAD}"

    deg = np.bincount(dst, minlength=N).astype(np.float64)  # in-degree
    core_of = np.searchsorted(cend, np.arange(N), side="right")

    # --- per-core node packing into NBLK blocks of TW, balancing lo+hi
    # incoming-edge counts per block (self edge included).
    src_is_lo_edge = core_of[src] <= 4
    lo_in = np.bincount(dst[src_is_lo_edge], minlength=N).astype(np.int64)
    hi_in = np.bincount(dst[~src_is_lo_edge], minlength=N).astype(np.int64)
    self_lo = (core_of <= 4).astype(np.int64)
    lo_in = lo_in + self_lo
    hi_in = hi_in + (1 - self_lo)

    glob_slot = np.zeros(N, np.int64)
    slot_of = {}
    for c in range(NC):
        nodes = np.arange(cstart[c], cend[c])
        n = len(nodes)
        tot = lo_in[nodes] + hi_in[nodes]
        order = np.argsort(-tot, kind="stable")
        loads = np.zeros(NBLK)
        cnts = np.zeros(NBLK, np.int64)
        blk = np.zeros(n, np.int64)
        pos = np.zeros(n, np.int64)
        for i in order:
            masked = np.where(cnts < TW, loads, np.inf)
            b = int(np.argmin(masked))
            blk[i] = b
            pos[i] = cnts[b]
            cnts[b] += 1
            loads[b] += tot[i]
        slots = blk * TW + pos
        # DRAM tables are partition-major: slot s lives at row
        # (s%128)*NTB + s//128 of the core's stripe.
        rows = (slots % 128) * NTB + slots // 128
        glob_slot[nodes] = c * NPAD + rows
        slot_of[c] = slots  # local slot of q-th protein-order node
        row_of_slot = rows  # noqa (kept for clarity)

    # --- per-(core, block, side) edge lists (incl. self edges)
    e_src_g = np.concatenate([glob_slot[src], glob_slot])           # + self
    e_dst = np.concatenate([dst, np.arange(N)])
    e_core = core_of[e_dst]
    e_row = glob_slot[e_dst] - e_core * NPAD
    e_slot = (e_row % NTB) * 128 + e_row // NTB   # row -> slot
    e_blk = e_slot // TW
    e_dcol = e_slot % TW
    # Edges with src row in [HI_BASE, LO_BOUND) may use either stream;
    # fill each block's lo side up to LO_CAP, spill the rest to hi.
    lo_only = e_src_g < HI_BASE
    flex = (~lo_only) & (e_src_g < LO_BOUND)
    e_lo = lo_only.copy()
    key = e_core * NBLK + e_blk
    order_f = np.flatnonzero(flex)
    order_f = order_f[np.argsort(key[order_f], kind="stable")]
    kf = key[order_f]
    grp_start = np.searchsorted(kf, np.arange(NC * NBLK))
    grp_end = np.searchsorted(kf, np.arange(NC * NBLK), side="right")
    nlo_only = np.bincount(key[lo_only], minlength=NC * NBLK)
    for g in range(NC * NBLK):
        room = max(0, LO_CAP - int(nlo_only[g]))
        take = min(room, grp_end[g] - grp_start[g])
        if take > 0:
            e_lo[order_f[grp_start[g]:grp_start[g] + take]] = True

    lo_cnt = np.zeros((NC, NBLK), np.int64)
    hi_cnt = np.zeros((NC, NBLK), np.int64)
    np.add.at(lo_cnt, (e_core[e_lo], e_blk[e_lo]), 1)
    np.add.at(hi_cnt, (e_core[~e_lo], e_blk[~e_lo]), 1)
    LO_T = int(np.ceil(lo_cnt.max() / 128))
    HI_T = int(np.ceil(hi_cnt.max() / 128))
    NT = NBLK * (LO_T + HI_T)

    per_core = []
    for c in range(NC):
        m = e_core == c
        cm_lo, cm_blk, cm_dcol, cm_srcg = e_lo[m], e_blk[m], e_dcol[m], e_src_g[m]
        order = np.lexsort((cm_dcol, ~cm_lo, cm_blk))  # group (blk, lo-first)
        cm_lo, cm_blk, cm_dcol, cm_srcg = (
            cm_lo[order], cm_blk[order], cm_dcol[order], cm_srcg[order])

        # dis in pi (packed-slot) order
        disp1 = np.ones(NPAD, np.float32)
        nodes = np.arange(cstart[c], cend[c])
        disp1[slot_of[c]] = (deg[nodes] + 1.0).astype(np.float32)
        dis = (1.0 / np.sqrt(disp1)).astype(np.float32)
        dis_nm = np.ascontiguousarray(dis.reshape(NTB, 128).T)  # [128, 50]

        cm_slot = cm_blk * TW + cm_dcol
        lo_idx = np.zeros(NBLK * LO_T * 128, np.int64)
        hi_idx = np.zeros(NBLK * HI_T * 128, np.int64)
        s_rows, s_cols, s_vals = [], [], []
        off = 0
        for b in range(NBLK):
            nl, nh = lo_cnt[c, b], hi_cnt[c, b]
            lo_sl = slice(off, off + nl)
            hi_sl = slice(off + nl, off + nl + nh)
            off += nl + nh
            lo_idx[b * LO_T * 128: b * LO_T * 128 + nl] = cm_srcg[lo_sl]
            hi_idx[b * HI_T * 128: b * HI_T * 128 + nh] = (
                cm_srcg[hi_sl] - HI_BASE)
            kt = b * (LO_T + HI_T)
            r = np.arange(nl)
            s_rows.append(r % 128)
            s_cols.append((kt + r // 128) * TW + cm_dcol[lo_sl])
            s_vals.append(dis[cm_slot[lo_sl]])
            r = np.arange(nh)
            s_rows.append(r % 128)
            s_cols.append((kt + LO_T + r // 128) * TW + cm_dcol[hi_sl])
            s_vals.append(dis[cm_slot[hi_sl]])
        s_all = np.zeros((128, NT * TW), bf16)
        s_all[np.concatenate(s_rows), np.concatenate(s_cols)] = (
            np.concatenate(s_vals))

        # x transposed into pi order
        x_pad = np.zeros((NPAD, D), np.float32)
        x_pad[slot_of[c]] = np.asarray(x[nodes], np.float32)
        x_t = np.ascontiguousarray(x_pad.T)             # [128, 6400]

        # readout masks/one-hots in pi (packed-slot) node-major order
        lens = pcounts[c * PPC:(c + 1) * PPC]
        starts = np.concatenate([[0], np.cumsum(lens)])[:-1]
        lysn = np.asarray(lysine_mask[nodes], np.float32)
        q = np.arange(ncore[c])
        pj = np.searchsorted(starts, q, side="right") - 1
        sl = slot_of[c]
        pone = np.zeros((128, NTB * PPC), bf16)
        pone[sl % 128, (sl // 128) * PPC + pj] = 1.0
        lys_nm = np.zeros((128, NTB), np.float32)
        lys_nm[sl % 128, sl // 128] = lysn

        per_core.append(dict(
            x_t=x_t,
            s_all=s_all,
            idx_lo=_pack_idx(lo_idx, NBLK * LO_T * 128),
            idx_hi=_pack_idx(hi_idx, NBLK * HI_T * 128),
            dis_nm=dis_nm,
            pone=pone,
            cnt_col=lens.astype(np.float32).reshape(PPC, 1),
            lys_nm=lys_nm,
        ))
    return per_core, LO_T, HI_T, NT


# ---------------------------------------------------------------- program

def _build_program(LO_T, HI_T, NT):
    tile_utils.max_sbuf_usage = 204 * 1024
    nc = bacc.Bacc("TRN2", target_bir_lowering=False, num_devices=NC,
                   num_swdge_queues=2)

    din = {}
    for name, shape, dt in [
        ("x_t", [D, NPAD], f32),
        ("s_all", [128, NT * TW], bft),
        ("idx_lo", [128, NBLK * LO_T * 8], i16),
        ("idx_hi", [128, NBLK * HI_T * 8], i16),
        ("dis_nm", [128, NTB], f32),
        ("pone", [128, NTB * PPC], bft),
        ("cnt_col", [PPC, 1], f32),
        ("lys_nm", [128, NTB], f32),
        ("convw", [D, L * D], f32),
        ("convb", [D, L], f32),
        ("attw", [D, 1], f32),
        ("outw", [D, 64], f32),
        ("outb", [64, 1], f32),
    ]:
        din[name] = nc.dram_tensor(name, shape, dt, kind="ExternalInput")
    out_t = nc.dram_tensor("out_t", [64, PPC], f32, kind="ExternalOutput")
    out_h = None
    if DBG_DUMP_H:
        out_h = nc.dram_tensor("out_h", [D, NPAD], f32, kind="ExternalOutput")

    LO_SLOTS = NBLK * LO_T * 128
    HI_SLOTS = NBLK * HI_T * 128

    with tile.TileContext(nc) as tc:
        with (
            tc.tile_pool(name="glob", bufs=1) as gp,
            tc.tile_pool(name="dram", bufs=1, space="DRAM") as dram,
        ):
            # resident SBUF state
            NCH = NPAD // 256
            htiles = [gp.tile([D, 256], f32, tag=f"h{i}", name=f"h{i}")
                      for i in range(NCH)]
            for i in range(NCH):
                nc.gpsimd.dma_start(
                    htiles[i][:], din["x_t"][:, i * 256:(i + 1) * 256])

            def h_col(c0, w):
                i = c0 // 256
                assert c0 % 256 + w <= 256
                return htiles[i][:, c0 % 256:c0 % 256 + w]
            dis_nm = gp.tile([128, NTB], f32)
            nc.gpsimd.dma_start(dis_nm[:], din["dis_nm"][:])
            idx_lo = gp.tile([128, LO_SLOTS // 16], i16)
            nc.gpsimd.dma_start(idx_lo[:], din["idx_lo"][:])
            idx_hi = gp.tile([128, HI_SLOTS // 16], i16)
            nc.gpsimd.dma_start(idx_hi[:], din["idx_hi"][:])
            convw = gp.tile([D, L * D], f32)
            nc.gpsimd.dma_start(convw[:], din["convw"][:])
            convb = gp.tile([D, L], f32)
            nc.gpsimd.dma_start(convb[:], din["convb"][:])

            stripe = dram.tile([NPAD, D], bft)
            hws_full = dram.tile([NPADG, D], bft)

            # ---------------- GCN layers
            with (
                tc.tile_pool(name="l_sb", bufs=1) as lp,
                tc.tile_pool(name="msgs", bufs=2) as mp,
                tc.tile_pool(name="eps", bufs=2) as ep,
                tc.tile_pool(name="ps_agg", bufs=4, space="PSUM") as ps_agg,
                tc.tile_pool(name="ps_big", bufs=2, space="PSUM") as ps_big,
                tc.tile_pool(name="ps_tr", bufs=2, space="PSUM") as ps_tr,
            ):
                s_sb = lp.tile([128, NT * TW], bft)
                nc.gpsimd.dma_start(s_sb[:], din["s_all"][:])

                def s_col(c0, w):
                    return s_sb[:, c0:c0 + w]
                hws_nm = [lp.tile([128, NTB // 5, 128], bft, tag=f"nm{i}",
                                  name=f"nm{i}") for i in range(5)]

                tident = lp.tile([128, 128], f32)
                make_identity(nc, tident[:])
                for layer in range(DBG_LAYERS):
                    # hw^T = W^T h^T (fp32), PE-transpose to node-major,
                    # then one DVE op scales by dis (src side) + casts bf16
                    for ch in range(NPAD // 256):
                        pw = ps_big.tile([D, 256], f32, tag="wmm")
                        nc.tensor.matmul(
                            out=pw[:],
                            lhsT=convw[:, layer * D:(layer + 1) * D],
                            rhs=htiles[ch][:], start=True, stop=True)
                        cb = ep.tile([D, 256], f32, tag="cb")
                        nc.vector.tensor_copy(cb[:], pw[:])
                        for i in range(2):
                            tb = ch * 2 + i
                            pt = ps_tr.tile([128, 128], f32, tag="ptr")
                            nc.tensor.transpose(
                                out=pt[:], in_=cb[:, i * 128:(i + 1) * 128],
                                identity=tident[:])
                            nc.vector.tensor_scalar_mul(
                                hws_nm[tb // (NTB // 5)][:, tb % (NTB // 5), :],
                                pt[:], dis_nm[:, tb:tb + 1])
                    spm = stripe[:].rearrange("(p k) f -> p k f", k=NTB)
                    for hf in range(5):
                        nc.gpsimd.dma_start(
                            spm[:, hf * (NTB // 5):(hf + 1) * (NTB // 5), :],
                            hws_nm[hf][:])
                    if DBG_NO_COLL:
                        nc.gpsimd.dma_start(hws_full[0:NPAD, :], stripe[:])
                    else:
                        nc.gpsimd.collective_compute(
                            "AllGather", mybir.AluOpType.bypass,
                            replica_groups=[list(range(NC))],
                            ins=[stripe.opt()], outs=[hws_full.opt()])

                    # gathers issued lazily in consumption order to avoid
                    # pool-slot deadlock; aggregate (dis[dst] folded into S)
                    # + relu epilogue
                    lo_chunks, hi_chunks = {}, {}

                    def get_chunk(done, ci, slots, idx, base_lo, tg):
                        gch = GCH // 2 if base_lo else GCH
                        if ci not in done:
                            s0 = ci * gch
                            n = min(gch, slots - s0)
                            m = mp.tile([128, gch // 128, 128], bft, tag=tg)
                            if DBG_NO_GATHER:
                                nc.vector.memset(m[:], 0.0)
                            else:
                                src_ap = (hws_full[HI_BASE:, :] if base_lo
                                          else hws_full[:])
                                nc.gpsimd.dma_gather(
                                    out_ap=m[:, : n // 128, :], in_ap=src_ap,
                                    idxs_ap=idx[:, s0 // 16:(s0 + n) // 16],
                                    num_idxs=n, num_idxs_reg=n, elem_size=D,
                                    single_packet=False)
                            done[ci] = m
                        return done[ci]

                    nt = LO_T + HI_T
                    for b in range(NBLK):
                        acc = ps_agg.tile([128, TW], f32, tag="agg")
                        for t in range(LO_T):
                            slot = b * LO_T * 128 + t * 128
                            mm = get_chunk(lo_chunks, slot // GCH, LO_SLOTS,
                                           idx_lo, False, "mlo")
                            col = (slot % GCH) // 128
                            k = b * nt + t
                            nc.tensor.matmul(
                                out=acc[:], lhsT=mm[:, col, :],
                                rhs=s_col(k * TW, TW),
                                start=(t == 0), stop=False)
                        for t in range(HI_T):
                            slot = b * HI_T * 128 + t * 128
                            mm = get_chunk(hi_chunks, slot // (GCH // 2),
                                           HI_SLOTS, idx_hi, True, "mhi")
                            col = (slot % (GCH // 2)) // 128
                            k = b * nt + LO_T + t
                            nc.tensor.matmul(
                                out=acc[:], lhsT=mm[:, col, :],
                                rhs=s_col(k * TW, TW),
                                start=False, stop=(t == HI_T - 1))
                        nc.scalar.activation(
                            h_col(b * TW, TW), acc[:], AF.Relu,
                            bias=convb[:, layer:layer + 1])

            if DBG_NO_READOUT:
                with tc.tile_pool(name="r0", bufs=1) as r0:
                    oz = r0.tile([64, PPC], f32)
                    nc.vector.tensor_copy(oz[:], htiles[0][0:64, 0:PPC])
                    nc.gpsimd.dma_start(out_t[:], oz[:])
            if DBG_DUMP_H:
                for i in range(NCH):
                    nc.gpsimd.dma_start(
                        out_h[:, i * 256:(i + 1) * 256], htiles[i][:])

            if not DBG_NO_READOUT:
                with (
                    tc.tile_pool(name="r_sb", bufs=1) as rp,
                    tc.tile_pool(name="r2", bufs=2) as rp2,
                    tc.tile_pool(name="ps_r", bufs=2, space="PSUM") as ps_r,
                    tc.tile_pool(name="ps_p", bufs=1, space="PSUM") as ps_p,
                ):
                    attw = rp.tile([D, 1], f32)
                    nc.gpsimd.dma_start(attw[:], din["attw"][:])
                    ident = rp.tile([128, 128], f32)
                    make_identity(nc, ident[:])
                    ones_r = rp.tile([1, 128], f32)
                    nc.vector.memset(ones_r[:], 1.0)

                    # scores in pi order -> DRAM bounce -> node-major cols
                    srow = rp.tile([1, NPAD], f32)
                    for ch in range(NPAD // 256):
                        pssc = ps_p.tile([1, 256], f32, tag="sc")
                        nc.tensor.matmul(
                            out=pssc[:], lhsT=attw[:],
                            rhs=htiles[ch][:], start=True, stop=True)
                        nc.vector.tensor_copy(
                            srow[:, ch * 256:(ch + 1) * 256], pssc[:])
                    row_scr = dram.tile([1, NPAD], f32)
                    nc.gpsimd.dma_start(row_scr[:], srow[:])
                    sc_nm = rp.tile([128, NTB], f32)
                    nc.gpsimd.dma_start(
                        sc_nm[:],
                        row_scr[:].rearrange("a (k p) -> p (a k)", p=128))

                    # global-shift masked softmax pieces, all in pi order.
                    # softmax is shift-invariant; a global max keeps exp <= 1.
                    gmax = rp.tile([1, 1], f32)
                    nc.vector.tensor_reduce(
                        out=gmax[:], in_=srow[:], axis=mybir.AxisListType.X,
                        op=mybir.AluOpType.max)
                    ngmax = rp.tile([1, 1], f32)
                    nc.vector.tensor_scalar_mul(ngmax[:], gmax[:], -1.0)
                    psng = ps_p.tile([128, 1], f32, tag="ng")
                    nc.tensor.matmul(
                        out=psng[:], lhsT=ones_r[:], rhs=ngmax[:],
                        start=True, stop=True)
                    ngcol = rp.tile([128, 1], f32)
                    nc.vector.tensor_copy(ngcol[:], psng[:])
                    exm = rp.tile([128, NTB], f32)
                    nc.scalar.activation(exm[:], sc_nm[:], AF.Exp, bias=ngcol[:])
                    lys_nm = rp.tile([128, NTB], f32)
                    nc.gpsimd.dma_start(lys_nm[:], din["lys_nm"][:])
                    nc.vector.tensor_mul(exm[:], exm[:], lys_nm[:])

                    # node-major h4 tiles: transpose+evict all tiles first
                    # (depends only on h), then fused pooling matmuls
                    # (rhs = [h | ex*h | ex], bf16) once exm is ready.
                    pone = rp.tile([128, NTB * PPC], bft)
                    nc.gpsimd.dma_start(pone[:], din["pone"][:])
                    hnm_all = rp.tile([128, NTB, 128], bft)
                    for tb in range(NTB):
                        pt = ps_r.tile([128, 128], f32, tag="tr")
                        nc.tensor.transpose(
                            out=pt[:], in_=h_col(tb * 128, 128),
                            identity=ident[:])
                        nc.vector.tensor_copy(hnm_all[:, tb, :], pt[:])
                    pall = ps_p.tile([PPC, 257], f32, tag="pall")
                    for tb in range(NTB):
                        rh = rp2.tile([128, 257], bft, tag="rh")
                        nc.vector.tensor_copy(rh[:, 0:128], hnm_all[:, tb, :])
                        nc.vector.tensor_scalar_mul(
                            rh[:, 128:256], hnm_all[:, tb, :],
                            exm[:, tb:tb + 1])
                        nc.vector.tensor_copy(
                            rh[:, 256:257], exm[:, tb:tb + 1])
                        nc.tensor.matmul(
                            out=pall[:],
                            lhsT=pone[:, tb * PPC:(tb + 1) * PPC], rhs=rh[:],
                            start=(tb == 0), stop=(tb == NTB - 1))

                    # c_j = 1/(max(cnt,1)*sqrt(cnt+1e-6)); rden = 1/max(dn,eps)
                    cnt = rp.tile([PPC, 1], f32)
                    nc.gpsimd.dma_start(cnt[:], din["cnt_col"][:])
                    cg = rp.tile([PPC, 1], f32)
                    nc.vector.tensor_scalar_max(cg[:], cnt[:], 1.0)
                    cnte = rp.tile([PPC, 1], f32)
                    nc.vector.tensor_scalar_add(cnte[:], cnt[:], 1.0e-6)
                    sq = rp.tile([PPC, 1], f32)
                    nc.scalar.activation(sq[:], cnte[:], AF.Sqrt)
                    mm_ = rp.tile([PPC, 1], f32)
                    nc.vector.tensor_mul(mm_[:], cg[:], sq[:])
                    cj = rp.tile([PPC, 1], f32)
                    nc.vector.reciprocal(cj[:], mm_[:])
                    dg = rp.tile([PPC, 1], f32)
                    nc.vector.tensor_scalar_max(
                        dg[:], pall[:, 256:257], 1.0e-30)
                    rden = rp.tile([PPC, 1], f32)
                    nc.vector.reciprocal(rden[:], dg[:])

                    pre = rp.tile([PPC, 128], f32)
                    nc.vector.tensor_scalar_mul(pre[:], pall[:, 0:128], cj[:])
                    lw = rp.tile([PPC, 128], f32)
                    nc.vector.tensor_scalar_mul(
                        lw[:], pall[:, 128:256], rden[:])
                    nc.vector.tensor_add(pre[:], pre[:], lw[:])

                    # out^T = outw^T @ pre^T + outb
                    ptp = ps_r.tile([128, 128], f32, tag="tr")
                    nc.tensor.transpose(
                        out=ptp[:, 0:PPC], in_=pre[:],
                        identity=ident[0:PPC, 0:PPC])
                    preT = rp.tile([128, PPC], f32)
                    nc.vector.tensor_copy(preT[:], ptp[:, 0:PPC])
                    outw = rp.tile([D, 64], f32)
                    nc.gpsimd.dma_start(outw[:], din["outw"][:])
                    outb = rp.tile([64, 1], f32)
                    nc.gpsimd.dma_start(outb[:], din["outb"][:])
                    pso = ps_p.tile([64, PPC], f32, tag="o")
                    nc.tensor.matmul(
                        out=pso[:], lhsT=outw[:], rhs=preT[:],
                        start=True, stop=True)
                    osb = rp.tile([64, PPC], f32)
                    nc.vector.tensor_scalar_add(osb[:], pso[:], outb[:])
                    nc.gpsimd.dma_start(out_t[:], osb[:])

    nc.compile()
    return nc


# ---------------------------------------------------------------- entry

def kernel(**inputs):
    x = np.asarray(inputs["x"], np.float32)
    edge_index = np.asarray(inputs["edge_index"])
    batch = np.asarray(inputs["batch"])
    lysine_mask = np.asarray(inputs["lysine_mask"])
    conv_w = np.asarray(inputs["conv_w"], np.float32)
    conv_b = np.asarray(inputs["conv_b"], np.float32)
    att_w = np.asarray(inputs["att_w"], np.float32)
    out_w = np.asarray(inputs["out_w"], np.float32)
    out_b = np.asarray(inputs["out_b"], np.float32)

    per_core, LO_T, HI_T, NT = _host_prep(x, edge_index, batch, lysine_mask)

    convw = np.ascontiguousarray(
        np.concatenate([conv_w[i] for i in range(L)], axis=1))  # [128, 512]
    convb = np.ascontiguousarray(
        np.stack([conv_b[i] for i in range(L)], axis=1))        # [128, 4]
    shared = dict(
        convw=convw, convb=convb,
        attw=att_w.reshape(D, 1).astype(np.float32),
        outw=out_w.astype(np.float32),
        outb=out_b.reshape(64, 1).astype(np.float32),
    )
    in_maps = []
    for c in range(NC):
        pc = per_core[c]
        in_maps.append({
            "x_t": pc["x_t"], "s_all": pc["s_all"],
            "idx_lo": pc["idx_lo"], "idx_hi": pc["idx_hi"],
            "dis_nm": pc["dis_nm"],
            "pone": pc["pone"], "cnt_col": pc["cnt_col"],
            "lys_nm": pc["lys_nm"], **shared,
        })

    nc_prog = _build_program(LO_T, HI_T, NT)
    trace = os.environ.get("GCN_TRACE", "") == "1"
    res = run_bass_kernel_spmd(
        nc_prog, in_maps, core_ids=list(range(NC)), trace=trace)
    if trace:
        import kernel as _self
        _self.LAST_RESULT = res
        print("HW exec time:", res.exec_time_ns, "ns")
    out = np.concatenate(
        [np.asarray(res.results[c]["out_t"], np.float32).T for c in range(NC)],
        axis=0)
    return out



# revision 14
# speedup vs baseline: 1.5694x; 1.5694x over previous
"""GCN message-passing kernel for Trainium2, 8 NeuronCores (SPMD).

Strategy (graph-parallel, fp8 messages):
- Nodes are protein-contiguous, sharded across 8 cores at protein boundaries
  (16 proteins/core, padded to 6400 nodes/core). Within a core, nodes are
  bin-packed into 50 blocks of 128 balancing incoming-edge counts; slot
  s = blk*128 + pos maps to partition pos, chunk blk everywhere (h, dis,
  table, S, pooling) so aggregation blocks coincide with node chunks.
- Message table is fp8e4m3 at 256B row stride with a 128B payload
  (row = dis[src] * (h @ W)[src]); gathers use elem_size=128/elem_step=256
  (bass's %256 payload assert is bypassed via direct InstDMAGatherAnt
  construction - verified byte-exact on hardware), halving per-edge DMA
  cost vs bf16.
- Aggregation is node-major: acc[dst,feat] = S^T @ msgs with S a 0/1
  one-hot (exact in fp8) via DoubleRow fp8 matmuls (256 slots each);
  dis[dst] is applied EXACTLY by the relu epilogue's per-partition ACT
  scale; conv bias enters through one bias slot per block whose S column
  is 1/dis[dst] and whose table row is conv_b (rewritten per layer at the
  reserved slot (pos 0, blk 49) of every core).
- h is kept bf16 both node-major (epilogue output; feeds readout) and
  feature-major (one PE transpose per chunk per layer; feeds the bf16
  h @ W matmuls computed directly node-major as lhsT=h_fm, rhs=W).
- Readout: scores via DVE mul+reduce against a broadcast att_w row,
  global-shift masked softmax, fused pooling matmuls with
  rhs = [h | ex*h | ex] per chunk, final projection per core.
"""
import os
import numpy as np
import ml_dtypes

DBG_LAYERS = int(os.environ.get("GCN_DBG_LAYERS", "4"))
DBG_NO_COLL = os.environ.get("GCN_DBG_NO_COLL", "") == "1"
DBG_NO_GATHER = os.environ.get("GCN_DBG_NO_GATHER", "") == "1"
DBG_NO_READOUT = os.environ.get("GCN_DBG_NO_READOUT", "") == "1"
DBG_DUMP_H = os.environ.get("GCN_DBG_DUMP_H", "") == "1"
DBG_DUMP_TB = os.environ.get("GCN_DBG_DUMP_TB", "") == "1"

import concourse.bacc as bacc
import concourse.tile as tile
import concourse.tile_utils as tile_utils
from concourse import mybir
from concourse.bass_utils import run_bass_kernel_spmd
from concourse.masks import make_identity

bf16 = ml_dtypes.bfloat16
E4 = ml_dtypes.float8_e4m3
AF = mybir.ActivationFunctionType

NC = 8
D = 128
L = 4
B = 128
PPC = B // NC          # proteins per core
NPAD = 6400            # padded nodes per core
NPADG = NC * NPAD      # global padded rows
NTB = NPAD // 128      # 50 chunks of 128 nodes == aggregation blocks
NBLK = NTB
TW = 128               # dst nodes per aggregation block
LO_BOUND = 32000       # lo gather covers rows [0, 32000)
HI_BASE = 18560        # hi gather covers rows [18560, 51200): 32639 <= int16
GCH = 8192             # gather slots per dma_gather instruction (64 cols)
BIAS_SLOT = NBLK - 1   # reserved slot (pos 0, blk 49) on every core

f32 = mybir.dt.float32
bft = mybir.dt.bfloat16
fp8 = mybir.dt.float8e4
i16 = mybir.dt.int16


# ---------------------------------------------------------------- host prep

def _pack_idx(vals, slots):
    """int16 gather index layout: position i -> partition i%16, col i//16,
    replicated across the 128 partitions."""
    assert len(vals) == slots and slots % 16 == 0
    arr = np.asarray(vals, np.int16).reshape(slots // 16, 16).T  # [16, s//16]
    return np.ascontiguousarray(np.tile(arr, (8, 1)))


def _even_ceil(x):
    t = int(np.ceil(x / 128))
    return t + (t % 2)


def _host_prep(x, edge_index, batch, lysine_mask):
    N = x.shape[0]
    src = np.asarray(edge_index[0], np.int64)
    dst = np.asarray(edge_index[1], np.int64)
    batch = np.asarray(batch, np.int64)

    pcounts = np.bincount(batch, minlength=B)
    pstart = np.concatenate([[0], np.cumsum(pcounts)])
    cstart = pstart[np.arange(NC) * PPC]
    cend = pstart[(np.arange(NC) + 1) * PPC]
    ncore = cend - cstart
    assert ncore.max() <= NPAD - 1, f"core node count {ncore.max()} > {NPAD-1}"
    assert pcounts.max() <= 128 * NTB

    deg = np.bincount(dst, minlength=N).astype(np.float64) + 1.0
    dis = (1.0 / np.sqrt(deg)).astype(np.float32)
    core_of = np.searchsorted(cend, np.arange(N), side="right")

    # --- per-core node packing into NBLK blocks of 128, balancing in-slot
    # (in-edges + self) counts per block; (pos 127, blk 49) is reserved.
    blk = np.zeros(N, np.int64)
    pos = np.zeros(N, np.int64)
    for c in range(NC):
        nodes = np.arange(cstart[c], cend[c])
        tot = deg[nodes]
        order = np.argsort(-tot, kind="stable")
        caps = np.full(NBLK, 128, np.int64)
        caps[NBLK - 1] = 127
        loads = np.zeros(NBLK)
        cnts = np.zeros(NBLK, np.int64)
        for i in order:
            masked = np.where(cnts < caps, loads, np.inf)
            b = int(np.argmin(masked))
            blk[nodes[i]] = b
            # (pos 0, blk 49) is the reserved bias slot on every core
            pos[nodes[i]] = cnts[b] + (1 if b == NBLK - 1 else 0)
            cnts[b] += 1
            loads[b] += tot[i]
    slot = blk * 128 + pos                    # local pi slot
    grow = core_of * NPAD + pos * NTB + blk   # global table row

    # --- edge list: real edges + self edges + one bias pseudo-edge per
    # (core, block) (dst col -1). Bias row is the reserved slot's row.
    e_src_row = np.concatenate([grow[src], grow])
    e_dst = np.concatenate([dst, np.arange(N)])
    e_core = core_of[e_dst]
    e_blk = blk[e_dst]
    e_col = pos[e_dst]
    bias_core = np.repeat(np.arange(NC), NBLK)
    bias_blk = np.tile(np.arange(NBLK), NC)
    bias_row_of_core = np.arange(NC) * NPAD + (NBLK - 1)  # (pos 0, blk 49)
    e_src_row = np.concatenate([e_src_row, bias_row_of_core[bias_core]])
    e_core = np.concatenate([e_core, bias_core])
    e_blk = np.concatenate([e_blk, bias_blk])
    e_col = np.concatenate([e_col, np.full(NC * NBLK, -1, np.int64)])

    cls = np.where(e_src_row < HI_BASE, 0,
                   np.where(e_src_row < LO_BOUND, 1, 2))
    key = e_core * NBLK + e_blk
    nl0 = np.bincount(key[cls == 0], minlength=NC * NBLK)
    nf = np.bincount(key[cls == 1], minlength=NC * NBLK)
    tot_cb = np.bincount(key, minlength=NC * NBLK)

    best = None
    for LO_T in range(_even_ceil(nl0.max()), _even_ceil(nl0.max()) + 8, 2):
        lo_fill = np.minimum(LO_T * 128, nl0 + nf)
        HI_T = _even_ceil((tot_cb - lo_fill).max())
        if best is None or LO_T + HI_T < best[0] + best[1]:
            best = (LO_T, HI_T)
    LO_T, HI_T = best
    NT = LO_T + HI_T

    per_core = []
    for c in range(NC):
        m = e_core == c
        rows_e, blk_e, col_e, cls_e = (
            e_src_row[m], e_blk[m], e_col[m], cls[m])
        order = np.lexsort((col_e, cls_e, blk_e))
        rows_e, blk_e, col_e, cls_e = (
            rows_e[order], blk_e[order], col_e[order], cls_e[order])
        bstart = np.searchsorted(blk_e, np.arange(NBLK))
        bend = np.searchsorted(blk_e, np.arange(NBLK), side="right")

        nodes = np.arange(cstart[c], cend[c])
        # dis / inv-dis in pi layout (pads -> 1 / 0)
        dis_nm = np.ones((128, NTB), np.float32)
        inv_nm = np.zeros((128, NTB), np.float32)
        dis_nm[pos[nodes], blk[nodes]] = dis[nodes]
        inv_nm[pos[nodes], blk[nodes]] = 1.0 / dis[nodes]

        lo_idx = np.zeros(NBLK * LO_T * 128, np.int64)
        hi_idx = np.zeros(NBLK * HI_T * 128, np.int64)  # already HI_BASE-offset
        s_all = np.zeros((128, NBLK * NT * 128), np.float32)
        for b in range(NBLK):
            sl = slice(bstart[b], bend[b])
            r_b, c_b, k_b = rows_e[sl], col_e[sl], cls_e[sl]
            n = len(r_b)
            n0 = int((k_b == 0).sum())
            nfb = int((k_b == 1).sum())
            take = min(LO_T * 128 - n0, nfb)
            assert take >= 0, f"block lo overflow {n0} > {LO_T*128}"
            nlo = n0 + take
            nhi = n - nlo
            assert nhi <= HI_T * 128
            for stream, cnt, off, idxarr, base_t, ibase in (
                (0, nlo, 0, lo_idx, 0, 0),
                (1, nhi, nlo, hi_idx, LO_T, HI_BASE),
            ):
                if cnt == 0:
                    continue
                rr = r_b[off:off + cnt] - ibase
                cc = c_b[off:off + cnt]
                T = LO_T if stream == 0 else HI_T
                idxarr[b * T * 128: b * T * 128 + cnt] = rr
                k = np.arange(cnt)
                p = k % 128
                t = base_t + k // 128
                scol = (b * NT + t) * 128
                real = cc >= 0
                s_all[p[real], scol[real] + cc[real]] = 1.0
                for j in np.flatnonzero(~real):  # bias slots (dense column)
                    s_all[p[j], scol[j]:scol[j] + 128] = inv_nm[:, b]

        x_t = np.zeros((D, NPAD), np.float32)
        x_t[:, slot[nodes]] = np.asarray(x[nodes], np.float32).T

        lens = pcounts[c * PPC:(c + 1) * PPC]
        starts = np.concatenate([[0], np.cumsum(lens)])[:-1]
        q = np.arange(ncore[c])
        pj = np.searchsorted(starts, q, side="right") - 1
        pone = np.zeros((128, NTB * PPC), bf16)
        pone[pos[nodes], blk[nodes] * PPC + pj] = 1.0
        lys_nm = np.zeros((128, NTB), np.float32)
        lys_nm[pos[nodes], blk[nodes]] = np.asarray(
            lysine_mask[nodes], np.float32)

        per_core.append(dict(
            x_t=x_t.astype(bf16),
            s_all=s_all.astype(E4),
            idx_lo=_pack_idx(lo_idx, NBLK * LO_T * 128),
            idx_hi=_pack_idx(hi_idx, NBLK * HI_T * 128),
            dis_nm=dis_nm,
            pone=pone,
            cnt_col=lens.astype(np.float32).reshape(PPC, 1),
            lys_nm=lys_nm,
        ))
    return per_core, LO_T, HI_T, NT


# ---------------------------------------------------------------- program

def _dma_gather_128(nc, out_ap, in_ap, idxs_ap, num_idxs):
    """dma_gather with a 128B payload on a 256B-stride table (elem_size=128
    fp8, elem_step=256). Bypasses bass's %256 payload assert; verified
    byte-exact on hardware."""
    g = nc.gpsimd
    _in_ap = g.lower_ap_dma(in_ap, for_custom_bir_dma=True)
    _idxs_ap = g.lower_ap(idxs_ap)
    _out_ap = g.lower_ap(out_ap)
    return g.add_instruction(mybir.InstDMAGatherAnt(
        name=g.bass.get_next_instruction_name(),
        ins=[*_in_ap, _idxs_ap, g.lower_val_access(g.to_reg(num_idxs))],
        outs=[_out_ap],
        transpose=False, num_idxs=num_idxs, elem_size=128,
        stride_bytes_256=1, gen_mode=0, single_packet=False,
        queue_num=0, sbuf_tokens_per_rank=0, sbuf_free_dim_per_rank=0,
        sbuf_free_dim_pad_per_rank=0, sbuf_byte_offset=0))


def _build_program(LO_T, HI_T, NT):
    tile_utils.max_sbuf_usage = 204 * 1024
    nc = bacc.Bacc("TRN2", target_bir_lowering=False, num_devices=NC,
                   num_swdge_queues=2)

    din = {}
    for name, shape, dt in [
        ("x_t", [D, NPAD], bft),
        ("s_all", [128, NBLK * NT * 128], fp8),
        ("idx_lo", [128, NBLK * LO_T * 8], i16),
        ("idx_hi", [128, NBLK * HI_T * 8], i16),
        ("dis_nm", [128, NTB], f32),
        ("pone", [128, NTB * PPC], bft),
        ("cnt_col", [PPC, 1], f32),
        ("lys_nm", [128, NTB], f32),
        ("convw", [D, L * D], bft),
        ("convb_pre", [128, L * D], fp8),
        ("attw_row", [1, D], f32),
        ("outw", [D, 64], f32),
        ("outb", [64, 1], f32),
    ]:
        din[name] = nc.dram_tensor(name, shape, dt, kind="ExternalInput")
    out_t = nc.dram_tensor("out_t", [64, PPC], f32, kind="ExternalOutput")
    out_h = None
    if DBG_DUMP_H:
        out_h = nc.dram_tensor("out_h", [128, NPAD], bft,
                               kind="ExternalOutput")
    out_tb = None
    if DBG_DUMP_TB:
        out_tb = nc.dram_tensor("out_tb", [128, NTB * 128], fp8,
                                kind="ExternalOutput")

    LO_SLOTS = NBLK * LO_T * 128
    HI_SLOTS = NBLK * HI_T * 128

    with tile.TileContext(nc) as tc:
        with (
            tc.tile_pool(name="glob", bufs=1) as gp,
            tc.tile_pool(name="dram", bufs=1, space="DRAM") as dram,
        ):
            # resident SBUF state
            h_fm = gp.tile([D, NPAD], bft, name="h_fm")
            nc.sync.dma_start(h_fm[:], din["x_t"][:])
            h_nm = gp.tile([128, NTB, 128], bft, name="h_nm")
            staging = gp.tile([128, NTB, 128], fp8, name="staging")
            s_sb = gp.tile([128, NBLK * NT * 128], fp8, name="s_sb")
            nc.sync.dma_start(s_sb[:], din["s_all"][:])
            dis_nm = gp.tile([128, NTB], f32)
            nc.sync.dma_start(dis_nm[:], din["dis_nm"][:])
            idx_lo = gp.tile([128, LO_SLOTS // 16], i16)
            nc.sync.dma_start(idx_lo[:], din["idx_lo"][:])
            idx_hi = gp.tile([128, HI_SLOTS // 16], i16)
            nc.sync.dma_start(idx_hi[:], din["idx_hi"][:])
            convw = gp.tile([D, L * D], bft)
            nc.sync.dma_start(convw[:], din["convw"][:])
            convb_pre = gp.tile([128, L * D], fp8)
            nc.sync.dma_start(convb_pre[:], din["convb_pre"][:])

            stripe = dram.tile([NPAD, 256], fp8)
            hws_full = dram.tile([NPADG, 256], fp8)
            tident = gp.tile([128, 128], bft)
            make_identity(nc, tident[:])

            # ---------------- GCN layers
            with (
                tc.tile_pool(name="msgs", bufs=2) as mp,
                tc.tile_pool(name="ps_w", bufs=2, space="PSUM") as ps_w,
                tc.tile_pool(name="ps_agg", bufs=4, space="PSUM") as ps_agg,
                tc.tile_pool(name="ps_tr", bufs=2, space="PSUM") as ps_tr,
            ):
                for layer in range(DBG_LAYERS):
                    # table: hws_nm = dis[src] * (h @ W), node-major fp8.
                    # pw_nm = h_fm_chunk^T @ W directly node-major.
                    for b in range(NTB):
                        pw = ps_w.tile([128, D], f32, tag="wmm")
                        nc.tensor.matmul(
                            out=pw[:],
                            lhsT=h_fm[:, b * 128:(b + 1) * 128],
                            rhs=convw[:, layer * D:(layer + 1) * D],
                            start=True, stop=True)
                        nc.scalar.activation(
                            staging[:, b, :], pw[:], AF.Copy,
                            scale=dis_nm[:, b:b + 1])
                    # bias table row at the reserved slot (pos 0, blk 49)
                    nc.vector.tensor_copy(
                        staging[0:1, NBLK - 1, :],
                        convb_pre[0:1, layer * D:(layer + 1) * D])
                    spm = stripe[:, 0:128].rearrange("(p k) f -> p k f", k=NTB)
                    nc.sync.dma_start(spm, staging[:])
                    if DBG_NO_COLL:
                        nc.gpsimd.dma_start(hws_full[0:NPAD, :], stripe[:])
                    else:
                        nc.gpsimd.collective_compute(
                            "AllGather", mybir.AluOpType.bypass,
                            replica_groups=[list(range(NC))],
                            ins=[stripe.opt()], outs=[hws_full.opt()])

                    # gathers issued lazily in consumption order; aggregate
                    # via DoubleRow fp8 matmuls; relu epilogue with exact
                    # dis[dst] as the ACT per-partition scale.
                    lo_chunks, hi_chunks = {}, {}

                    def get_chunk(done, ci, slots, idx, base_hi, tg):
                        if ci not in done:
                            s0 = ci * GCH
                            n = min(GCH, slots - s0)
                            m = mp.tile([128, GCH // 128, 128], fp8, tag=tg)
                            if DBG_NO_GATHER:
                                nc.vector.memset(m[:], 0.0)
                            else:
                                src_ap = (hws_full[HI_BASE:, 0:128] if base_hi
                                          else hws_full[:, 0:128])
                                _dma_gather_128(
                                    nc, m[:, : n // 128, :], src_ap,
                                    idx[:, s0 // 16:(s0 + n) // 16], n)
                            done[ci] = m
                        return done[ci]

                    for b in range(NBLK):
                        acc = ps_agg.tile([128, D], f32, tag="agg")
                        for j in range(NT // 2):
                            if 2 * j < LO_T:
                                col = b * LO_T + 2 * j
                                mm = get_chunk(
                                    lo_chunks, col // (GCH // 128), LO_SLOTS,
                                    idx_lo, False, "mlo")
                            else:
                                col = b * HI_T + 2 * (j - LO_T // 2)
                                mm = get_chunk(
                                    hi_chunks, col // (GCH // 128), HI_SLOTS,
                                    idx_hi, True, "mhi")
                            cc = col % (GCH // 128)
                            sc0 = (b * NT + 2 * j) * 128
                            nc.tensor.matmul(
                                out=acc[:],
                                lhsT=s_sb[:, sc0:sc0 + 256].rearrange(
                                    "p (i d) -> p i d", i=2),
                                rhs=mm[:, cc:cc + 2, :],
                                start=(j == 0), stop=(j == NT // 2 - 1),
                                perf_mode=mybir.MatmulPerfMode.DoubleRow)
                        nc.scalar.activation(
                            h_nm[:, b, :], acc[:], AF.Relu,
                            scale=dis_nm[:, b:b + 1])
                        if layer < DBG_LAYERS - 1 or DBG_DUMP_H:
                            pt = ps_tr.tile([128, 128], bft, tag="ptr")
                            nc.tensor.transpose(
                                out=pt[:], in_=h_nm[:, b, :],
                                identity=tident[:])
                            nc.vector.tensor_copy(
                                h_fm[:, b * 128:(b + 1) * 128], pt[:])

            if DBG_DUMP_H:
                for b in range(NTB):
                    nc.gpsimd.dma_start(
                        out_h[:, b * 128:(b + 1) * 128],
                        h_fm[:, b * 128:(b + 1) * 128])
            if DBG_DUMP_TB:
                nc.gpsimd.dma_start(
                    out_tb[:].rearrange("p (k f) -> p k f", k=NTB),
                    staging[:])

            if DBG_NO_READOUT:
                with tc.tile_pool(name="r0", bufs=1) as r0:
                    oz = r0.tile([64, PPC], f32)
                    nc.vector.tensor_copy(oz[:], h_nm[0:64, 0, 0:PPC])
                    nc.gpsimd.dma_start(out_t[:], oz[:])

            if not DBG_NO_READOUT:
                with (
                    tc.tile_pool(name="r_sb", bufs=1) as rp,
                    tc.tile_pool(name="r2", bufs=2) as rp2,
                    tc.tile_pool(name="ps_r", bufs=2, space="PSUM") as ps_r,
                    tc.tile_pool(name="ps_p", bufs=1, space="PSUM") as ps_p,
                ):
                    ident = rp.tile([128, 128], f32)
                    make_identity(nc, ident[:])
                    ones_r = rp.tile([1, 128], f32)
                    nc.vector.memset(ones_r[:], 1.0)
                    attw = rp.tile([1, D], f32)
                    nc.sync.dma_start(attw[:], din["attw_row"][:])

                    # att_w broadcast to all partitions (ones outer product)
                    psat = ps_r.tile([128, D], f32, tag="tr")
                    nc.tensor.matmul(out=psat[:], lhsT=ones_r[:],
                                     rhs=attw[:], start=True, stop=True)
                    attrep = rp.tile([128, D], bft)
                    nc.vector.tensor_copy(attrep[:], psat[:])

                    # scores node-major via DVE mul + row-reduce
                    sc_nm = rp.tile([128, NTB], f32)
                    for t in range(NTB):
                        tmp = rp2.tile([128, D], bft, tag="sc")
                        nc.vector.tensor_mul(tmp[:], h_nm[:, t, :], attrep[:])
                        nc.vector.tensor_reduce(
                            out=sc_nm[:, t:t + 1], in_=tmp[:],
                            axis=mybir.AxisListType.X, op=mybir.AluOpType.add)

                    # global-shift masked softmax pieces (shift-invariant)
                    colmax = rp.tile([128, 1], f32)
                    nc.vector.tensor_reduce(
                        out=colmax[:], in_=sc_nm[:],
                        axis=mybir.AxisListType.X, op=mybir.AluOpType.max)
                    ptm = ps_r.tile([128, 128], f32, tag="tr")
                    nc.tensor.transpose(
                        out=ptm[0:1, :], in_=colmax[:], identity=ident[:])
                    rowmax = rp.tile([1, 128], f32)
                    nc.vector.tensor_copy(rowmax[:], ptm[0:1, :])
                    gmax = rp.tile([1, 1], f32)
                    nc.vector.tensor_reduce(
                        out=gmax[:], in_=rowmax[:],
                        axis=mybir.AxisListType.X, op=mybir.AluOpType.max)
                    ngmax = rp.tile([1, 1], f32)
                    nc.vector.tensor_scalar_mul(ngmax[:], gmax[:], -1.0)
                    psng = ps_p.tile([128, 1], f32, tag="ng")
                    nc.tensor.matmul(out=psng[:], lhsT=ones_r[:],
                                     rhs=ngmax[:], start=True, stop=True)
                    ngcol = rp.tile([128, 1], f32)
                    nc.vector.tensor_copy(ngcol[:], psng[:])
                    exm = rp.tile([128, NTB], f32)
                    nc.scalar.activation(exm[:], sc_nm[:], AF.Exp,
                                         bias=ngcol[:])
                    lys_nm = rp.tile([128, NTB], f32)
                    nc.sync.dma_start(lys_nm[:], din["lys_nm"][:])
                    nc.vector.tensor_mul(exm[:], exm[:], lys_nm[:])

                    # fused pooling matmuls: rhs = [h | ex*h | ex]
                    pone = rp.tile([128, NTB * PPC], bft)
                    nc.sync.dma_start(pone[:], din["pone"][:])
                    pall = ps_p.tile([PPC, 257], f32, tag="pall")
                    for t in range(NTB):
                        rh = rp2.tile([128, 257], bft, tag="rh")
                        nc.vector.tensor_copy(rh[:, 0:128], h_nm[:, t, :])
                        nc.vector.tensor_scalar_mul(
                            rh[:, 128:256], h_nm[:, t, :], exm[:, t:t + 1])
                        nc.vector.tensor_copy(
                            rh[:, 256:257], exm[:, t:t + 1])
                        nc.tensor.matmul(
                            out=pall[:],
                            lhsT=pone[:, t * PPC:(t + 1) * PPC], rhs=rh[:],
                            start=(t == 0), stop=(t == NTB - 1))

                    # c_j = 1/(max(cnt,1)*sqrt(cnt+1e-6)); rden = 1/max(dn,eps)
                    cnt = rp.tile([PPC, 1], f32)
                    nc.sync.dma_start(cnt[:], din["cnt_col"][:])
                    cg = rp.tile([PPC, 1], f32)
                    nc.vector.tensor_scalar_max(cg[:], cnt[:], 1.0)
                    cnte = rp.tile([PPC, 1], f32)
                    nc.vector.tensor_scalar_add(cnte[:], cnt[:], 1.0e-6)
                    sq = rp.tile([PPC, 1], f32)
                    nc.scalar.activation(sq[:], cnte[:], AF.Sqrt)
                    mm_ = rp.tile([PPC, 1], f32)
                    nc.vector.tensor_mul(mm_[:], cg[:], sq[:])
                    cj = rp.tile([PPC, 1], f32)
                    nc.vector.reciprocal(cj[:], mm_[:])
                    dg = rp.tile([PPC, 1], f32)
                    nc.vector.tensor_scalar_max(
                        dg[:], pall[:, 256:257], 1.0e-30)
                    rden = rp.tile([PPC, 1], f32)
                    nc.vector.reciprocal(rden[:], dg[:])

                    pre = rp.tile([PPC, 128], f32)
                    nc.vector.tensor_scalar_mul(pre[:], pall[:, 0:128], cj[:])
                    lw = rp.tile([PPC, 128], f32)
                    nc.vector.tensor_scalar_mul(
                        lw[:], pall[:, 128:256], rden[:])
                    nc.vector.tensor_add(pre[:], pre[:], lw[:])

                    # out^T = outw^T @ pre^T + outb
                    ptp = ps_r.tile([128, 128], f32, tag="tr")
                    nc.tensor.transpose(
                        out=ptp[:, 0:PPC], in_=pre[:],
                        identity=ident[0:PPC, 0:PPC])
                    preT = rp.tile([128, PPC], f32)
                    nc.vector.tensor_copy(preT[:], ptp[:, 0:PPC])
                    outw = rp.tile([D, 64], f32)
                    nc.sync.dma_start(outw[:], din["outw"][:])
                    outb = rp.tile([64, 1], f32)
                    nc.sync.dma_start(outb[:], din["outb"][:])
                    pso = ps_p.tile([64, PPC], f32, tag="o")
                    nc.tensor.matmul(
                        out=pso[:], lhsT=outw[:], rhs=preT[:],
                        start=True, stop=True)
                    osb = rp.tile([64, PPC], f32)
                    nc.vector.tensor_scalar_add(osb[:], pso[:], outb[:])
                    nc.gpsimd.dma_start(out_t[:], osb[:])

    nc.compile()
    return nc


# ---------------------------------------------------------------- entry

def kernel(**inputs):
    x = np.asarray(inputs["x"], np.float32)
    edge_index = np.asarray(inputs["edge_index"])
    batch = np.asarray(inputs["batch"])
    lysine_mask = np.asarray(inputs["lysine_mask"])
    conv_w = np.asarray(inputs["conv_w"], np.float32)
    conv_b = np.asarray(inputs["conv_b"], np.float32)
    att_w = np.asarray(inputs["att_w"], np.float32)
    out_w = np.asarray(inputs["out_w"], np.float32)
    out_b = np.asarray(inputs["out_b"], np.float32)

    per_core, LO_T, HI_T, NT = _host_prep(x, edge_index, batch, lysine_mask)

    convw = np.ascontiguousarray(
        np.concatenate([conv_w[i] for i in range(L)], axis=1)).astype(bf16)
    convb_pre = np.tile(
        np.concatenate([conv_b[i] for i in range(L)]).astype(E4), (128, 1))
    shared = dict(
        convw=convw, convb_pre=convb_pre,
        attw_row=att_w.reshape(1, D).astype(np.float32),
        outw=out_w.astype(np.float32),
        outb=out_b.reshape(64, 1).astype(np.float32),
    )
    in_maps = []
    for c in range(NC):
        pc = per_core[c]
        in_maps.append({
            "x_t": pc["x_t"], "s_all": pc["s_all"],
            "idx_lo": pc["idx_lo"], "idx_hi": pc["idx_hi"],
            "dis_nm": pc["dis_nm"],
            "pone": pc["pone"], "cnt_col": pc["cnt_col"],
            "lys_nm": pc["lys_nm"], **shared,
        })

    nc_prog = _build_program(LO_T, HI_T, NT)
    trace = os.environ.get("GCN_TRACE", "") == "1"
    res = run_bass_kernel_spmd(
        nc_prog, in_maps, core_ids=list(range(NC)), trace=trace)
    if trace:
        import kernel as _self
        _self.LAST_RESULT = res
        print("HW exec time:", res.exec_time_ns, "ns")
    out = np.concatenate(
        [np.asarray(res.results[c]["out_t"], np.float32).T for c in range(NC)],
        axis=0)
    return out
